# revision 1
# baseline (speedup 1.0000x reference)
"""Bass/Trainium2 kernel for nn_Attn_51127290691658.

Reference computation (S=1024, B=64, H=512):
    cat    = concat([broadcast(hidden), encoder_outputs], -1)   [S,B,2H]
    energy = tanh(cat @ W_attn.T + b_attn)                      [S,B,H]
    scores = energy @ beta                                      [S,B,1]
    out    = softmax(scores.transpose(0,2,1), axis=0)           [S,1,B]

Decomposition (W1 = W_attn[:, :H], W2 = W_attn[:, H:]):
    U[b,h]       = W1[h,:] . hidden[b,:] + b_attn[h]       (tiny)
    energyT[h,s] = tanh(W2 @ E_b^T + U[:,b])   per batch b (big)
    score[b,s]   = beta . tanh_energy[:, s]
    out[s,b]     = softmax over s

Sharding: data-parallel over B across 8 cores (8 batch elements/core);
W_attn/b_attn/beta replicated. Softmax is local per batch element.

Data path: all E transposes run on the DMA XBAR (dma_start
transpose=True on the sync HWDGE ring) instead of the PE, freeing
~30us of TensorE time. E streams per b-pair on two load rings (SWDGE
fp32->fp16 cast for 3/4 of each pair, scalar HWDGE fp32 + DVE cast for
the rest), each pair in its own SBUF buffer. Tile's DMA-transpose
deadlock guard serializes every XBAR against all prior DMAs; a
post-pass (_strip_guard_waits) rewrites the XBAR/load semaphore waits
down to the true data deps, and _pace_loads throttles the SWDGE loads
so the XBAR stream keeps SBUF-fabric headroom. The beta-dot uses 4
col-tiled concurrent matmuls (tile_position=(0,32hc)) into disjoint
PSUM partition groups, ~4x faster than selector matmuls, with the
cross-group sum on DVE. fp16 PE matmuls with fp32 PSUM accumulation;
tanh(energy + U) fused on ScalarE; softmax uses a fixed exp offset
(scores bounded); final [s,b] transpose fused with the 1/sum scaling
by streaming diag(scale) through the PE; PE warmup/filler matmuls
bridge load latency and the softmax tail so the HAM clock stays at
2.4 GHz. Measured on trn2: ~128-137us across 8 cores (vs 132.6us for
the previous PE-transpose version); absmax error vs the fp32
reference ~3.9e-3 (fp16 input quantization dominates).
"""

import sys
import types

import numpy as np

S, B, H = 1024, 64, 512
NCORES = 8
BC = B // NCORES  # 8 batch elements per core
KC = H // 128     # 4 contraction chunks
HC = H // 128     # 4 output h chunks
SGS = 2           # two 512-wide s groups
SG = S // SGS     # 512
SI = S // 128     # 8 s chunks of 128
SJ = SI // SGS    # 4 s chunks per group
EP16_BUFS = 7     # c16 buffers; only the last pair reuses a slot


def _install_axon_hooks_shim():
    """The container image's `antenv` lacks `axon_hooks`; without it,
    run_bass_kernel_spmd(trace=True) cannot find the NTFF hook. Register a
    minimal in-memory module and install the ctypes-based hook if available.
    Harmless when tracing is not requested."""
    try:
        import antenv
    except ImportError:
        return
    if "antenv.axon_hooks" in sys.modules:
        return
    mod = types.ModuleType("antenv.axon_hooks")
    mod._hook = None

    def set_axon_ntff_profile_hook(h):
        mod._hook = h

    def get_axon_ntff_profile_hook():
        return mod._hook

    mod.set_axon_ntff_profile_hook = set_axon_ntff_profile_hook
    mod.get_axon_ntff_profile_hook = get_axon_ntff_profile_hook
    sys.modules["antenv.axon_hooks"] = mod
    antenv.axon_hooks = mod
    try:
        from trn_agent_boot.trn_boot import _ntff_profile_via_ctypes

        hook = _ntff_profile_via_ctypes("/opt/axon/libaxon_pjrt.so")
        if hook is not None:
            set_axon_ntff_profile_hook(hook)
    except Exception:
        pass


_install_axon_hooks_shim()

import os  # noqa: E402

import concourse.bass as bass  # noqa: E402
import concourse.mybir as mybir  # noqa: E402
import concourse.tile as tile  # noqa: E402
from concourse.bass_utils import run_bass_kernel_spmd  # noqa: E402
from concourse.masks import make_identity  # noqa: E402

F32 = mybir.dt.float32
F16 = mybir.dt.float16


def _split_waits(nc, max_waits=1):
    """The walrus build in this container encodes at most one sem-wait per
    instruction ("Too many sync wait commands" otherwise). Tile emits up to
    ~5. Splitting excess waits into preceding same-engine NoOps is
    semantically identical (engine queues execute in order)."""
    ctr = 0
    for fn in nc.m.functions:
        for blk in fn.blocks:
            insts = list(blk.instructions)
            new = []
            changed = False
            for inst in insts:
                si = inst.sync_info
                if si is not None and len(si.on_wait) > max_waits:
                    waits = list(si.on_wait)
                    keep = waits[-max_waits:]
                    extra = waits[:-max_waits]
                    for i in range(0, len(extra), max_waits):
                        ctr += 1
                        new.append(
                            mybir.InstNoOp(
                                name=f"WSPLIT-{id(nc) & 0xFFFF}-{ctr}",
                                engine=inst.engine,
                                bass_nofuse=True,
                                sync_info=mybir.SyncInfo(
                                    on_wait=extra[i : i + max_waits], on_update=[]
                                ),
                            )
                        )
                    inst.sync_info = mybir.SyncInfo(
                        on_wait=keep, on_update=list(si.on_update)
                    )
                    changed = True
                new.append(inst)
            if changed:
                try:
                    blk.instructions = new
                except Exception:
                    blk.instructions.clear()
                    blk.instructions.extend(new)


def _dedupe_ldw(nc):
    """Remove back-to-back InstLdweights (per engine stream) that reload the
    exact same weights AP: the PE array keeps the stationary operand between
    matmuls, so a reload is pure overhead. Waits move to the next instruction."""

    def key(inst):
        a = inst.ins[0]
        return (a.memref, a.offset, str(a.ap), str(a.dtype))

    n = 0
    for fn in nc.m.functions:
        for blk in fn.blocks:
            insts = list(blk.instructions)
            last_w = {}
            drop = set()
            pend_waits = {}
            new = []
            for inst in insts:
                eng = getattr(inst, "engine", None)
                nm = type(inst).__name__
                if nm == "InstLdweights":
                    k = key(inst)
                    if last_w.get(str(eng)) == k:
                        si = inst.sync_info
                        if si is not None and (si.on_wait or si.on_update):
                            pend_waits.setdefault(str(eng), []).append(si)
                        n += 1
                        continue
                    last_w[str(eng)] = k
                elif nm == "InstMatmult":
                    pw = pend_waits.pop(str(eng), None)
                    if pw:
                        si = inst.sync_info
                        ow = [w for p in pw for w in p.on_wait] + (
                            list(si.on_wait) if si else []
                        )
                        ou = (list(si.on_update) if si else []) + [
                            u for p in pw for u in p.on_update
                        ]
                        inst.sync_info = mybir.SyncInfo(on_wait=ow, on_update=ou)
                new.append(inst)
            if n:
                try:
                    blk.instructions = new
                except Exception:
                    blk.instructions.clear()
                    blk.instructions.extend(new)
    return n


def _strip_guard_waits(nc, deps):
    """Tile's DMA-transpose deadlock guard serializes every XBAR transpose
    against ALL previously scheduled DMAs on every ring (and loads against
    prior transposes), forcing a ~12us pipeline beat. The actual HW hazard
    is only a transpose running CONCURRENTLY with another SBUF->SBUF DMA;
    here all transposes sit on the one sync HWDGE queue (engine-serialized)
    and every other DMA is DRAM<->SBUF, so the guard waits are vacuous.

    For each instruction name in `deps` (E-path loads and XBARs), rewrite
    its DMA-semaphore waits: keep only waits covering the instruction's
    REAL producer set (threshold lowered to that producer's cumulative sem
    value); engine-semaphore waits (tile's genuine data/anti deps) are kept
    untouched."""
    n_drop = n_low = n_add = 0
    for fn in nc.m.functions:
        for blk in fn.blocks:
            semcum = {}
            producers = {}  # sem -> list[(cum_after, inst_name)]
            prodinfo = {}  # inst_name -> (sem_id, sem_name, cum_after)
            for inst in blk.instructions:
                si = inst.sync_info
                name = getattr(inst, "name", None)
                if si is not None and name in deps:
                    allowed = deps[name]
                    new_waits = []
                    for w in si.on_wait:
                        sem = w.ant_name or ""
                        if not (sem.startswith("DMASW") or sem.startswith("DMAHW")):
                            new_waits.append(w)
                            continue
                        hits = [
                            c
                            for c, pn in producers.get(sem, [])
                            if pn in allowed and c <= w.wait_value
                        ]
                        if hits:
                            if max(hits) < w.wait_value:
                                w.wait_value = max(hits)
                                n_low += 1
                            new_waits.append(w)
                        else:
                            n_drop += 1
                    # tile may have expressed a producer dep only through a
                    # (dropped) coalesced guard wait: re-add explicit waits
                    # for every allowed producer not already covered.
                    for pn in allowed:
                        if pn not in prodinfo:
                            continue
                        sid, sem, cum = prodinfo[pn]
                        if not any(
                            (w.ant_name or "") == sem and w.wait_value >= cum
                            for w in new_waits
                        ):
                            new_waits.append(
                                mybir.SyncWait(
                                    sync_type="semaphore",
                                    id=sid,
                                    ant_name=sem,
                                    wait_mode="sem-ge-imm",
                                    wait_value=cum,
                                    wait_reg=None,
                                )
                            )
                            n_add += 1
                    inst.sync_info = mybir.SyncInfo(
                        on_wait=new_waits, on_update=list(si.on_update)
                    )
                    si = inst.sync_info
                if si is not None:
                    for u in si.on_update:
                        sem = u.ant_name or ""
                        if sem.startswith("DMASW") or sem.startswith("DMAHW"):
                            semcum[sem] = semcum.get(sem, 0) + (
                                u.update_value or 0
                            )
                            producers.setdefault(sem, []).append(
                                (semcum[sem], getattr(inst, "name", None))
                            )
                            prodinfo[getattr(inst, "name", None)] = (
                                u.id,
                                sem,
                                semcum[sem],
                            )
    return n_drop, n_low


def _reorder_xbars(nc, rec):
    """The tile list-scheduler may emit the sync queue's XBARs out of pair
    order (e.g. three g0 XBARs before pair0's g1), stalling the first mains.
    Rewrite the XBAR slots in the block so they execute in canonical
    (pair, g) order; sem waits/updates ride with each instruction and remain
    valid under reordering (counters are monotone, data deps explicit)."""
    order = {}
    for (pi, g), name in rec["xb"].items():
        order[name] = (pi, g)
    for fn in nc.m.functions:
        for blk in fn.blocks:
            insts = blk.instructions
            pos = [
                k
                for k, i in enumerate(insts)
                if type(i).__name__ == "InstDmaTransposeAnt"
                and getattr(i, "name", None) in order
            ]
            if not pos:
                continue
            xs = sorted((insts[k] for k in pos), key=lambda i: order[i.name])
            new = list(insts)
            for k, x in zip(pos, xs):
                new[k] = x
            try:
                blk.instructions = new
            except Exception:
                blk.instructions.clear()
                blk.instructions.extend(new)


def _pace_loads(nc, rec, depth=4):
    """Throttle the big SWDGE pair loads so the load flood leaves SBUF
    fabric headroom for the XBAR stream: ldg0(pi) waits until the XBAR
    `depth` slots before its consumer xbar(pi,0) in the SCHEDULED sync
    stream has completed. Pacing on a strictly stream-earlier XBAR is
    cycle-free regardless of the scheduler's chosen order."""
    xb_names = set(rec["xb"].values())
    npairs = len(rec["ldg0"])
    for fn in nc.m.functions:
        for blk in fn.blocks:
            semcum = {}
            xbar_seq = []  # stream order: (name, sem_id, ant_name, cum)
            for inst in blk.instructions:
                si = inst.sync_info
                if si is None:
                    continue
                for u in si.on_update:
                    sem = u.ant_name or ""
                    if sem.startswith("DMASW") or sem.startswith("DMAHW"):
                        semcum[sem] = semcum.get(sem, 0) + (u.update_value or 0)
                        if getattr(inst, "name", None) in xb_names:
                            xbar_seq.append(
                                (inst.name, u.id, sem, semcum[sem])
                            )
            if not xbar_seq:
                continue
            xpos = {name: k for k, (name, _, _, _) in enumerate(xbar_seq)}
            for inst in blk.instructions:
                name = getattr(inst, "name", None)
                for pi in range(npairs):
                    if name != rec["ldg0"][pi]:
                        continue
                    k = xpos.get(rec["xb"][(pi, 0)], 0) - depth
                    if k < 0:
                        continue
                    _, sid, sem, cum = xbar_seq[k]
                    si = inst.sync_info
                    ow = list(si.on_wait) if si else []
                    ow.append(
                        mybir.SyncWait(
                            sync_type="semaphore",
                            id=sid,
                            ant_name=sem,
                            wait_mode="sem-ge-imm",
                            wait_value=cum,
                            wait_reg=None,
                        )
                    )
                    inst.sync_info = mybir.SyncInfo(
                        on_wait=ow,
                        on_update=list(si.on_update) if si else [],
                    )


def build_nc(split=True):
    nc = bass.Bass()
    enc = nc.dram_tensor("enc", [S, BC, H], F32, kind="ExternalInput")
    hid = nc.dram_tensor("hid", [BC, H], F32, kind="ExternalInput")
    w_attn = nc.dram_tensor("w_attn", [H, 2 * H], F32, kind="ExternalInput")
    b_attn = nc.dram_tensor("b_attn", [H], F32, kind="ExternalInput")
    beta = nc.dram_tensor("beta", [H, 1], F32, kind="ExternalInput")
    out = nc.dram_tensor("out", [128, SI, BC], F32, kind="ExternalOutput")

    rec = {"ldg0": {}, "ldg1": {}, "xb": {}}
    with tile.TileContext(nc) as tc:
        _body(tc, enc, hid, w_attn, b_attn, beta, out, rec)
    if os.environ.get("BASS_STRIP_GUARD", "1") == "1":
        # Strip guard waits from XBARs (real deps — own g0 load / DVE cast —
        # are provably present as explicit waits) and from loads (with one
        # c16 buffer per pair there is no slot reuse, so loads have NO
        # legitimate DMA-sem deps; c32 slot anti-deps ride DVE engine sems,
        # which the pass preserves).
        # All E buffers are unique (no slot reuse): loads have no legitimate
        # DMA-sem deps at all; XBAR g0 needs only its own SWDGE load, g1 only
        # the DVE cast (engine sem, preserved by the pass).
        deps = {}
        npairs = len(rec["ldg0"])
        for pi in range(npairs):
            deps[rec["xb"][(pi, 0)]] = {rec["ldg0"][pi]}
            deps[rec["xb"][(pi, 1)]] = set()
            deps[rec["ldg0"][pi]] = set()
            deps[rec["ldg1"][pi]] = set()
        nd, nl = _strip_guard_waits(nc, deps)
        _pace_loads(nc, rec, depth=int(os.environ.get("BASS_PACE_DEPTH", "4")))
    if os.environ.get("BASS_DEDUPE_LDW", "1") == "1":
        _dedupe_ldw(nc)
    if split:
        _split_waits(nc, max_waits=1)
    return nc


def _body(tc, enc, hid, w_attn, b_attn, beta, out, rec):
    nc = tc.nc
    import contextlib

    with contextlib.ExitStack() as ctx:
        const = ctx.enter_context(tc.tile_pool(name="const", bufs=1))
        cpool = ctx.enter_context(tc.tile_pool(name="cpool", bufs=1))
        etp = ctx.enter_context(tc.tile_pool(name="etp", bufs=1))
        thp = ctx.enter_context(tc.tile_pool(name="thp", bufs=3))
        pse = ctx.enter_context(tc.tile_pool(name="pse", bufs=4, space="PSUM"))
        pssp = ctx.enter_context(tc.tile_pool(name="pssp", bufs=2, space="PSUM"))
        psm = ctx.enter_context(tc.tile_pool(name="psm", bufs=1, space="PSUM"))

        Tanh = mybir.ActivationFunctionType.Tanh
        Exp = mybir.ActivationFunctionType.Exp

        # ---------------- PE warmup ----------------
        # Keep TensorE busy from t~1us so the HAM clock gate flips to
        # 2.4 GHz before the real matmul phase (needs ~3.4us sustained).
        identw = const.tile([128, 128], F32)
        make_identity(nc, identw)
        wps = pse.tile([128, SG], F32, tag="pe", name="wps")
        warm_pre = int(os.environ.get("BASS_WARMUP_PRE", "60"))
        warm_post = int(os.environ.get("BASS_WARMUP_POST", "16"))
        for _ in range(warm_pre):
            nc.tensor.transpose(wps[:, :128], identw, identw)

        ident16 = const.tile([128, 128], F16)
        make_identity(nc, ident16)
        wfill = const.tile([128, SG], F16)
        nc.vector.memset(wfill, 0.125)

        def pe_filler(n):
            # N=512 fp16 matmuls into the warmup psum tile: keeps the PE
            # busy (HAM clock at 2.4 GHz) while waiting on loads/softmax
            for _ in range(n):
                nc.tensor.matmul(wps, ident16, wfill, start=True, stop=True)
        ident8 = const.tile([BC, BC], F32)
        make_identity(nc, ident8)

        # small const loads on the sync HWDGE queue (SWDGE queue stays free
        # so the W pieces start transferring as early as possible)
        beta32 = const.tile([128, KC], F32)
        with nc.allow_non_contiguous_dma(reason="512-element strided constant load"):
            nc.sync.dma_start(
                out=beta32, in_=beta.rearrange("(c p) o -> p (c o)", p=128)
            )
        betat = const.tile([128, KC], F16)
        nc.vector.tensor_copy(out=betat, in_=beta32)
        hid32 = const.tile([BC, H], F32)
        nc.sync.dma_start(out=hid32, in_=hid[:, :])
        hid16 = const.tile([BC, H], F16)
        nc.vector.tensor_copy(out=hid16, in_=hid32)
        batt = const.tile([128, HC], F32)
        with nc.allow_non_contiguous_dma(reason="512-element strided constant load"):
            nc.sync.dma_start(out=batt, in_=b_attn.rearrange("(c p) -> p c", p=128))

        # transposed E tiles, one per b-pair: et[(j,sg)][k0, sj, bb, kc, s0]
        #   = E[sg*512 + sj*128 + s0, 2j+bb, kc*128 + k0]
        # Written by 2048-wide XBAR ops ([128, 2sj, 2bb, KC, 128] halves),
        # the size where the XBAR hits its best ~276 GB/s engine-rate.
        et = {}
        for j in range(BC // 2):
            for sg in range(SGS):
                et[(j, sg)] = etp.tile(
                    [128, SJ, 2, KC, 128], F16, tag=f"et{j}_{sg}", name=f"et{j}_{sg}"
                )

        # W: fp16 cast inside the SWDGE DMA (queued behind pair0 g0),
        # PE-transposed during the warmup window into the stationary layout.
        # wtt[k0, hc, half, kc, h0] = W_attn[hc*128+h0, half*H + kc*128 + k0]
        wt16 = const.tile([128, HC, 2 * H], F16)
        wtt = const.tile([128, HC, 2, KC, 128], F16)

        def load_w():
            wre = w_attn.rearrange("(ho p) k -> p ho k", p=128)
            nc.gpsimd.dma_start(out=wt16[:, :, :], in_=wre)

        def setup_w():
            for ho in range(HC):
                for half in range(2):
                    ps = psm.tile([128, KC * 128], F16, tag="w16")
                    for kc in range(KC):
                        nc.tensor.transpose(
                            ps[:, kc * 128 : (kc + 1) * 128],
                            wt16[
                                :, ho, half * H + kc * 128 : half * H + (kc + 1) * 128
                            ],
                            ident16,
                        )
                    nc.vector.tensor_copy(
                        out=wtt[:, ho, half, :, :],
                        in_=ps.rearrange("p (kc h) -> p kc h", kc=KC),
                    )

        hidt = const.tile([128, KC, BC], F16)
        u_sb = const.tile([128, HC, BC], F32)

        def setup_hidt():
            for kc in range(KC):
                ps = psm.tile([128, KC * 128], F16, tag="w16", name="hidtr")[:, :BC]
                nc.tensor.transpose(
                    ps, hid16[:, kc * 128 : (kc + 1) * 128], ident16[:BC, :BC]
                )
                nc.vector.tensor_copy(out=hidt[:, kc, :], in_=ps)

        def setup_u():
            # U[h, b] = W1[h, :] . hidden[b, :] + b_attn[h]
            for hc in range(HC):
                psu = psm.tile([128, BC], F32, tag="sc")
                for kc in range(KC):
                    nc.tensor.matmul(
                        psu,
                        wtt[:, hc, 0, kc, :],
                        hidt[:, kc, :],
                        start=(kc == 0),
                        stop=(kc == KC - 1),
                    )
                nc.vector.tensor_scalar_add(u_sb[:, hc, :], psu, batt[:, hc : hc + 1])

        # beta selector matrices: bsel[k, b, hc, col] = beta[hc*128+k] iff col==b
        bsel = const.tile([128, BC, KC, BC], F16)
        nc.vector.memset(bsel, 0.0)
        for b in range(BC):
            for hc in range(HC):
                nc.vector.tensor_copy(
                    out=bsel[:, b, hc, b : b + 1], in_=betat[:, hc : hc + 1]
                )

        # scores are bounded well inside fp32 exp range (|score| < ~70 for
        # randn inputs, exp overflows at 88), so softmax uses a fixed offset
        # instead of a max-reduction; both halves share it, so no rescale.
        nbias = const.tile([BC, 1], F32)
        nc.vector.memset(nbias, -45.0)
        e_sb = const.tile([BC, SGS, SG], F32)
        sc_sb = const.tile([BC, SGS, SG], F32)
        sc_h = const.tile([BC, SGS, SG], F32)
        t0 = const.tile([BC, 1], F32)

        def load_pair(sg, j):
            # one 2MB b-pair = [128 s0, 4 sj, 2 b, 512 h], split across BOTH
            # load rings: sj 0-2 cast fp32->fp16 inside the SWDGE DMA, sj 3
            # loads fp32 on the scalar HWDGE ring and casts on VectorE.
            # Every pair gets its OWN buffers (no slot reuse -> no anti-deps
            # -> the deadlock-guard strip below is race-free by construction).
            pi = sg * (BC // 2) + j
            c16 = cpool.tile([128, SJ, 2, H], F16, tag=f"c16_{pi}")
            src = enc.rearrange("(sg sj p) b k -> p sg sj b k", p=128, sg=SGS)
            h0 = nc.gpsimd.dma_start(
                out=c16[:, 0:3, :, :], in_=src[:, sg, 0:3, 2 * j : 2 * j + 2, :]
            )
            c32 = cpool.tile([128, 1, 2, H], F32, tag=f"c32_{pi}")
            h1 = nc.scalar.dma_start(
                out=c32, in_=src[:, sg, 3:4, 2 * j : 2 * j + 2, :]
            )
            nc.vector.tensor_copy(out=c16[:, 3:4, :, :], in_=c32)
            rec["ldg0"][pi] = h0.ins.name
            rec["ldg1"][pi] = h1.ins.name
            return c16

        def xbar_group(sg, j, g, c16):
            # XBARs: g0 = 3072-wide (sj 0-2), g1 = 1024-wide (sj 3):
            # [128 s0, (sjj,bb,k)] -> et[k0, (sjj, bb, kc), s0]
            lo, hi = (0, 3) if g == 0 else (3, 4)
            h = nc.sync.dma_start(
                out=et[(j, sg)][:, lo:hi, :, :, :],
                in_=c16[:, lo:hi, :, :],
                transpose=True,
            )
            rec["xb"][(sg * (BC // 2) + j, g)] = h.ins.name

        def mains(b, sg, th):
            j, bb = divmod(b, 2)
            for hc in range(HC):
                pe = pse.tile([128, SG], F32, tag="pe", name=f"pe{b % 2}")
                for kc in range(KC):
                    nc.tensor.matmul(
                        pe,
                        wtt[:, hc, 1, kc, :],
                        et[(j, sg)][:, :, bb, kc, :],
                        start=(kc == 0),
                        stop=(kc == KC - 1),
                    )
                # tanh(energy + U[:, b]) fused on ScalarE, fp16 out
                nc.scalar.activation(
                    out=th[:, hc, :],
                    in_=pe,
                    func=Tanh,
                    bias=u_sb[:, hc, b : b + 1],
                    scale=1.0,
                )

        def beta_mms(b, sg, th, pss):
            # 4 col-tiled matmuls run concurrently on disjoint 32-col groups
            # of the PE array; group hc accumulates its h-block's partial
            # scores over b into PSUM partitions [32hc, 32hc+8).
            for hc in range(HC):
                nc.tensor.matmul(
                    pss[32 * hc : 32 * hc + BC, :],
                    bsel[:, b, hc, :],
                    th[:, hc, :],
                    start=(b == 0),
                    stop=(b == BC - 1),
                    tile_position=(0, 32 * hc),
                )

        def score_head(sg, pss):
            # cross-group sum on DVE (one PSUM operand per op), then exp
            nc.vector.tensor_copy(out=sc_h[:, sg, :], in_=pss[0:BC, :])
            nc.vector.tensor_add(sc_h[:, sg, :], sc_h[:, sg, :], pss[32 : 32 + BC, :])
            nc.vector.tensor_add(sc_h[:, sg, :], sc_h[:, sg, :], pss[64 : 64 + BC, :])
            nc.vector.tensor_add(
                sc_sb[:, sg, :], sc_h[:, sg, :], pss[96 : 96 + BC, :]
            )
            nc.scalar.activation(
                out=e_sb[:, sg, :], in_=sc_sb[:, sg, :], func=Exp, bias=nbias,
                scale=1.0,
            )

        # ---------------- main pipeline ----------------
        # Flat sequence over the 8 (sg, b-pair) units. Pair loads stay
        # LOOKAHEAD units ahead (rings stream 2MB DMAs back-to-back at
        # ~260 GB/s); XBAR transposes chase each pair's data on the sync
        # engine; main matmuls trail one pair; col-tiled beta matmuls trail
        # one b; W transposes + U setup run under the PE warmup.
        load_w()
        pairs = [(sg, j) for sg in range(SGS) for j in range(BC // 2)]
        cgs = {}
        nload = 0
        LOOKAHEAD = 1
        for k in range(min(LOOKAHEAD + 1, len(pairs))):
            cgs[pairs[k]] = load_pair(*pairs[k])
            nload = k + 1
        setup_hidt()
        setup_w()
        setup_u()
        pe_filler(warm_post)

        pss = {}
        ths = {}
        for sg in range(SGS):
            pss[sg] = pssp.tile([128, SG], F32, tag="pss", name=f"pss{sg}")

        def run_b(sg, b):
            ths[(sg, b)] = thp.tile(
                [128, HC, SG], F16, tag="th", name=f"th{sg}_{b}"
            )
            mains(b, sg, ths[(sg, b)])
            if b > 0:
                beta_mms(b - 1, sg, ths[(sg, b - 1)], pss[sg])

        for idx, (sg, j) in enumerate(pairs):
            while nload < min(idx + LOOKAHEAD + 1, len(pairs)):
                cgs[pairs[nload]] = load_pair(*pairs[nload])
                nload += 1
            for g in range(2):
                xbar_group(sg, j, g, cgs[(sg, j)])
            cgs.pop((sg, j))
            if idx >= 1:
                psg, pj = pairs[idx - 1]
                run_b(psg, 2 * pj)
                run_b(psg, 2 * pj + 1)
                if psg == 0 and pj == BC // 2 - 1:
                    beta_mms(BC - 1, 0, ths[(0, BC - 1)], pss[0])
            if idx == 5:
                # exp/sum of the first half, hidden under sg1's matmuls
                score_head(0, pss[0])
                nc.vector.reduce_sum(t0, e_sb[:, 0, :], axis=mybir.AxisListType.X)
        run_b(1, BC - 2)
        run_b(1, BC - 1)
        beta_mms(BC - 1, 1, ths[(1, BC - 1)], pss[1])
        # keep the clock at 2.4 GHz while the softmax head chain runs
        pe_filler(int(os.environ.get("BASS_TAIL_FILL", "0")))

        # ---------------- softmax tail (second half + normalize) --------
        osb = const.tile([128, SI, BC], F32)
        score_head(1, pss[1])
        sm = const.tile([BC, 1], F32)
        nc.vector.reduce_sum(sm, e_sb[:, 1, :], axis=mybir.AxisListType.X)
        nc.vector.tensor_add(sm, sm, t0)
        rp = const.tile([BC, 1], F32)
        nc.vector.reciprocal(rp, sm)
        # D = diag(1/sum): transpose-and-normalize in one PE op per chunk:
        # out[s, b] = sum_k e[k, s] * D[k, b] = e[b, s] / sum_b
        dmat = const.tile([BC, BC], F32)
        nc.vector.tensor_scalar_mul(dmat, ident8, rp)
        for si in range(SI):
            sg = si // SJ
            sj = si % SJ
            po = psm.tile([128, BC], F32, tag="sc")
            nc.tensor.matmul(
                po,
                e_sb[:, sg, sj * 128 : (sj + 1) * 128],
                dmat,
                start=True,
                stop=True,
            )
            nc.vector.tensor_copy(out=osb[:, si, :], in_=po)
        # contiguous store; host reshapes [p, si, b] -> [si*128+p, b]
        nc.sync.dma_start(out=out[:, :, :], in_=osb)


_NC_CACHE = None


def _get_nc():
    global _NC_CACHE
    if _NC_CACHE is None:
        _NC_CACHE = build_nc()
    return _NC_CACHE


def run(inputs, trace=False, **kw):
    """Shard, execute on 8 NeuronCores, gather. Returns (output, BassKernelResults)."""
    hidden = np.asarray(inputs["hidden"], dtype=np.float32)
    enc = np.ascontiguousarray(np.asarray(inputs["encoder_outputs"], dtype=np.float32))
    w_attn = np.ascontiguousarray(np.asarray(inputs["W_attn"], dtype=np.float32))
    b_attn = np.ascontiguousarray(np.asarray(inputs["b_attn"], dtype=np.float32))
    beta = np.ascontiguousarray(np.asarray(inputs["beta"], dtype=np.float32))

    nc = _get_nc()
    in_maps = []
    for c in range(NCORES):
        b0 = c * BC
        in_maps.append(
            {
                "enc": np.ascontiguousarray(enc[:, b0 : b0 + BC, :]),
                "hid": np.ascontiguousarray(hidden[0, b0 : b0 + BC, :]),
                "w_attn": w_attn,
                "b_attn": b_attn,
                "beta": beta,
            }
        )
    res = run_bass_kernel_spmd(
        nc, in_maps, core_ids=list(range(NCORES)), trace=trace, **kw
    )
    outs = [
        np.transpose(res.results[c]["out"], (1, 0, 2)).reshape(S, BC)
        for c in range(NCORES)
    ]
    full = np.concatenate(outs, axis=1)  # [S, B]
    return full[:, None, :].astype(np.float32), res  # [S, 1, B]


def kernel(**inputs):
    out, _ = run(inputs, trace=False)
    return out

